# revision 6
# baseline (speedup 1.0000x reference)
"""EvolveGCN-O on 8 Trainium2 NeuronCores (Bass/Tile).

Contract: kernel(**inputs) takes the FULL unsharded inputs of
nn_EvolveGCNO (T=6, N=50000, E=800000, F=H=128, CH=307, n_classes=2)
and returns the FULL (N, 2) float32 output.

Algorithmic reduction (exact): the reference's LSTM weight-evolution is
data-independent (the cell input is the weight matrix itself, zero initial
h/c each step), and the output reads only h1[T-1], which depends only on
h0[T-1].  Everything collapses to the LAST graph snapshot:
    W0 = lstm0^6(W00);  W1 = lstm1^6(W01)
    h0p = (feats[5] * rsqrt(max(deg_out,1))) @ W0
    h0  = rrelu(segment_sum(h0p[src], dst) * rsqrt(max(deg_in,1)))
    h1p = (h0 * rsqrt(deg_out)) @ W1
    h1  = rrelu(segment_sum(h1p[src], dst) * rsqrt(deg_in))
    out = relu(h1 @ mlp_w1 + mlp_b1) @ mlp_w2 + mlp_b2

Distribution over 8 cores (all arithmetic on-device):
  * node positions are permuted: dst nodes are dealt to cores by in-degree
    (snake), each core's slice padded to nloc_pad rows (pad rows are zero).
    Tables (h0p, h1p) live in DRAM in this position order on every core.
  * stage A: every core computes the full h0p table (replicated) — feats are
    staged feature-partitioned/permuted by the host; LSTM weight evolution,
    degree rsqrt, projection and scaling run on-device.
  * segment-sum: per-core rounds of dma_gather row-fetches (int16 indices
    force an A/B table-half split at B0; within-degree-group nodes are
    ordered by A-half in-degree so both halves' rounds stay dense; padded
    lanes fetch a known zero row) + DVE accumulation into per-group SBUF
    aggregates.
  * one AllGather exchanges projected h1p slices between the two convs.
  * per-core MLP tail; output [2, nloc_pad] per core; host reassembles.
"""

import os
import numpy as np

import concourse.bass as bass
import concourse.bacc as bacc
import concourse.tile as tile
from concourse import mybir
from concourse.masks import make_identity
from concourse import bass_utils

FP = mybir.dt.float32
I16 = mybir.dt.int16

RRELU_SLOPE = float((1.0 / 8.0 + 1.0 / 3.0) / 2.0)

N_CORES = 8
F = H = 128
CH, NCLS = 307, 2


# ----------------------------------------------------------------------------
# Host-side schedule
# ----------------------------------------------------------------------------

def _build_schedule(src, dst, n_nodes, n_cores, grp_blocks=7, force_cores_a=None):
    N = n_nodes
    assert N % n_cores == 0
    nloc = N // n_cores
    # at least one zero pad row per core slice
    nblk = nloc // 128 + 1 if nloc % 128 == 0 else (nloc + 127) // 128
    nloc_pad = nblk * 128
    NPOS = n_cores * nloc_pad
    ngrp = (nblk + grp_blocks - 1) // grp_blocks

    cores_a = min(n_cores, 32768 // nloc_pad)
    if force_cores_a is not None:
        cores_a = force_cores_a
    assert cores_a >= 1 and (n_cores - cores_a) * nloc_pad <= 32768
    B0 = cores_a * nloc_pad

    deg_out = np.bincount(src, minlength=N).astype(np.int64)
    deg_in = np.bincount(dst, minlength=N).astype(np.int64)

    order = np.argsort(-deg_in, kind="stable")
    ranks = np.arange(N)
    blk = ranks // n_cores
    pos_in_blk = ranks % n_cores
    core_of_rank = np.where(blk % 2 == 0, pos_in_blk, n_cores - 1 - pos_in_blk)
    ln0 = np.empty((n_cores, nloc), np.int64)
    for k in range(n_cores):
        ln0[k] = order[core_of_rank == k]
    core_of_node = np.empty(N, np.int64)
    for k in range(n_cores):
        core_of_node[ln0[k]] = k

    edgeA = core_of_node[src] < cores_a
    degA_all = np.bincount(dst[edgeA], minlength=N).astype(np.int64)

    # final local order: within degree-groups, sort by A-half in-degree desc
    local_nodes = np.empty((n_cores, nloc), np.int64)
    for k in range(n_cores):
        ln = ln0[k].copy()
        for g in range(ngrp):
            q0 = g * grp_blocks * 128
            q1 = min(q0 + grp_blocks * 128, nloc)
            if q0 >= nloc:
                break
            seg = ln[q0:q1]
            ln[q0:q1] = seg[np.argsort(-degA_all[seg], kind="stable")]
        local_nodes[k] = ln
    pos_of_node = np.empty(N, np.int64)
    for k in range(n_cores):
        pos_of_node[local_nodes[k]] = k * nloc_pad + np.arange(nloc)

    PAD_LOC = nloc  # known zero row (within half-local coordinates)

    def make_csr(mask, base):
        d = dst[mask]
        s = src[mask]
        eo = np.argsort(d, kind="stable")
        st = np.zeros(N + 1, np.int64)
        np.cumsum(np.bincount(d, minlength=N), out=st[1:])
        return st, pos_of_node[s[eo]] - base

    stA, valA = make_csr(edgeA, 0)
    stB, valB = make_csr(~edgeA, B0)
    degB_all = deg_in - degA_all

    def padded_mat(st, vals, deg):
        md = int(deg.max()) if N else 0
        md = max(md, 1)
        pm = np.full((n_cores, nloc, md), PAD_LOC, np.int64)
        for k in range(n_cores):
            ln = local_nodes[k]
            dl = deg[ln]
            tot = int(dl.sum())
            if tot:
                cum0 = np.concatenate([[0], np.cumsum(dl)[:-1]])
                within = np.arange(tot) - np.repeat(cum0, dl)
                flat = np.repeat(st[ln], dl) + within
                rows = np.repeat(np.arange(nloc), dl)
                pm[k, rows, within] = vals[flat]
        return pm

    pmA = padded_mat(stA, valA, degA_all)
    pmB = padded_mat(stB, valB, degB_all)
    dA = np.stack([degA_all[local_nodes[k]] for k in range(n_cores)])
    dB = np.stack([degB_all[local_nodes[k]] for k in range(n_cores)])

    rounds = []   # (half, g, r, js, je, soff)
    soff = 0
    lanes_tot = 0
    for g in range(ngrp):
        q0 = g * grp_blocks * 128
        q1 = min(q0 + grp_blocks * 128, nloc)
        if q0 >= nloc:
            break
        for half, dH in (("A", dA), ("B", dB)):
            seg = dH[:, q0:q1]
            R = int(seg.max())
            for r in range(R):
                part = seg > r  # [cores, q1-q0]
                js_, je_ = nblk, 0
                for k in range(n_cores):
                    nz = np.nonzero(part[k])[0]
                    if nz.size:
                        js_ = min(js_, int(nz[0]) // 128)
                        je_ = max(je_, int(nz[-1]) // 128 + 1)
                if je_ <= js_:
                    continue
                rounds.append((half, g, r, js_, je_, soff))
                soff += (je_ - js_) * 8
                lanes_tot += (je_ - js_) * 128
    ctot16 = soff

    total_edges_pc = int(dA.sum(1).max() + dB.sum(1).max())
    overhead = lanes_tot / max(1, total_edges_pc)

    idx = np.full((n_cores, 128, ctot16), PAD_LOC, np.int16)
    for k in range(n_cores):
        for (half, g, r, js_, je_, so) in rounds:
            q0 = g * grp_blocks * 128
            lanes = (je_ - js_) * 128
            v = np.full(lanes, PAD_LOC, np.int64)
            lo = q0 + js_ * 128
            hi = min(q0 + je_ * 128, nloc)
            if hi > lo:
                pm, dH = (pmA, dA) if half == "A" else (pmB, dB)
                seg = pm[k, lo:hi, r].copy()
                seg[dH[k, lo:hi] <= r] = PAD_LOC
                v[: hi - lo] = seg
            S = lanes // 16
            m = v.reshape(S, 16).T.astype(np.int16)
            idx[k, :, so:so + S] = np.tile(m, (128 // 16, 1))

    def local_deg_tiles(deg):
        t = np.ones((n_cores, 128, nblk), np.float32)
        for k in range(n_cores):
            dpad = np.ones(nloc_pad, np.float32)
            dpad[:nloc] = deg[local_nodes[k]].astype(np.float32)
            t[k] = dpad.reshape(nblk, 128).T
        return t

    return dict(
        local_nodes=local_nodes, pos_of_node=pos_of_node,
        rounds=rounds, ctot16=ctot16, lanes_overhead=overhead,
        idx=idx,
        deg_in_loc=local_deg_tiles(deg_in), deg_out_loc=local_deg_tiles(deg_out),
        deg_out_full=deg_out,
        nloc=nloc, nblk=nblk, nloc_pad=nloc_pad, NPOS=NPOS, B0=B0,
        ngrp=ngrp, grp_blocks=grp_blocks,
        n_nodes=N, n_cores=n_cores,
    )


# ----------------------------------------------------------------------------
# Bass program
# ----------------------------------------------------------------------------

def _build_program(sched, gather_bufs=4, debug=False):
    N = sched["n_nodes"]
    n_cores = sched["n_cores"]
    nloc = sched["nloc"]
    nblk = sched["nblk"]
    GB = sched["grp_blocks"]
    ngrp = sched["ngrp"]
    ctot16 = sched["ctot16"]
    rounds = sched["rounds"]
    nloc_pad = sched["nloc_pad"]
    NPOS = sched["NPOS"]
    B0 = sched["B0"]
    nblk_glob = NPOS // 128

    nc = bacc.Bacc("TRN2", target_bir_lowering=False, debug=False,
                   num_devices=n_cores)

    featsT = nc.dram_tensor("featsT", [F, NPOS], FP, kind="ExternalInput")
    idx_d = nc.dram_tensor("idx", [128, ctot16], I16, kind="ExternalInput")
    deg_out_g = nc.dram_tensor("deg_out_g", [128, nblk_glob], FP, kind="ExternalInput")
    deg_in_l = nc.dram_tensor("deg_in_l", [128, nblk], FP, kind="ExternalInput")
    deg_out_l = nc.dram_tensor("deg_out_l", [128, nblk], FP, kind="ExternalInput")
    w00_d = nc.dram_tensor("w00", [H, H], FP, kind="ExternalInput")
    w01_d = nc.dram_tensor("w01", [H, H], FP, kind="ExternalInput")
    wih0_d = nc.dram_tensor("wih0", [H, 4 * H], FP, kind="ExternalInput")
    wih1_d = nc.dram_tensor("wih1", [H, 4 * H], FP, kind="ExternalInput")
    bih0_d = nc.dram_tensor("bih0", [128, 4 * H], FP, kind="ExternalInput")
    bhh0_d = nc.dram_tensor("bhh0", [128, 4 * H], FP, kind="ExternalInput")
    bih1_d = nc.dram_tensor("bih1", [128, 4 * H], FP, kind="ExternalInput")
    bhh1_d = nc.dram_tensor("bhh1", [128, 4 * H], FP, kind="ExternalInput")
    w1_d = nc.dram_tensor("mlp_w1", [H, CH], FP, kind="ExternalInput")
    b1_d = nc.dram_tensor("mlp_b1", [128, 3], FP, kind="ExternalInput")
    w2_d = nc.dram_tensor("mlp_w2", [128, 3 * NCLS], FP, kind="ExternalInput")
    b2_d = nc.dram_tensor("mlp_b2", [NCLS, 1], FP, kind="ExternalInput")

    out_d = nc.dram_tensor("out", [NCLS, nloc_pad], FP, kind="ExternalOutput")
    if debug:
        dbg_h0p = nc.dram_tensor("dbg_h0p", [NPOS, F], FP, kind="ExternalOutput")
        dbg_agg0 = nc.dram_tensor("dbg_agg0", [128, nloc_pad], FP, kind="ExternalOutput")
        dbg_agin = nc.dram_tensor("dbg_agin", [nloc_pad, F], FP, kind="ExternalOutput")
        dbg_agg1 = nc.dram_tensor("dbg_agg1", [128, nloc_pad], FP, kind="ExternalOutput")

    ch_sizes = [128, 128, CH - 256]
    grp_cols = [128 * (min((g + 1) * GB, nblk) - g * GB) for g in range(ngrp)]

    with tile.TileContext(nc) as tc:
        with (
            tc.tile_pool(name="const", bufs=1) as cst,
            tc.tile_pool(name="main", bufs=1) as main,
            tc.tile_pool(name="stream", bufs=3) as stream,
            tc.tile_pool(name="gather", bufs=gather_bufs) as gpool,
            tc.tile_pool(name="psA", bufs=4, space="PSUM") as psA,
            tc.tile_pool(name="psB", bufs=2, space="PSUM") as psB,
            tc.tile_pool(name="psC", bufs=2, space="PSUM") as psC,
            tc.tile_pool(name="dram", bufs=1, space="DRAM") as dpool,
        ):
            ident = cst.tile([128, 128], FP)
            make_identity(nc, ident[:])

            # ---------------- LSTM weight evolution --------------------------
            def lstm_chain(w_init_d, wih_d, bih_dd, bhh_dd, tag):
                wih_rs = cst.tile([128, 4 * H], FP, tag=f"wih{tag}")
                nc.sync.dma_start(out=wih_rs[:], in_=wih_d[:])
                wihT = cst.tile([128, 4 * H], FP, tag=f"wihT{tag}")
                for b in range(4):
                    pt = psA.tile([128, 128], FP, tag="t128")
                    nc.tensor.transpose(out=pt[:], in_=wih_rs[:, b * 128:(b + 1) * 128], identity=ident[:])
                    nc.vector.tensor_copy(out=wihT[:, b * 128:(b + 1) * 128], in_=pt[:])
                bt = cst.tile([128, 4 * H], FP, tag=f"b{tag}")
                bload = stream.tile([128, 4 * H], FP, tag="bload")
                nc.sync.dma_start(out=bt[:], in_=bih_dd[:])
                nc.sync.dma_start(out=bload[:], in_=bhh_dd[:])
                nc.vector.tensor_add(out=bt[:], in0=bt[:], in1=bload[:])

                w_cur = cst.tile([128, H], FP, tag=f"wcur{tag}")
                nc.sync.dma_start(out=w_cur[:], in_=w_init_d[:])
                for _ in range(6):
                    pt = psA.tile([128, 128], FP, tag="t128")
                    nc.tensor.transpose(out=pt[:], in_=w_cur[:], identity=ident[:])
                    wT = stream.tile([128, 128], FP, tag="lstm_wT")
                    nc.vector.tensor_copy(out=wT[:], in_=pt[:])
                    gates = psB.tile([128, 4 * H], FP, tag="mm512")
                    nc.tensor.matmul(out=gates[:], lhsT=wT[:], rhs=wihT[:], start=True, stop=True)
                    nc.vector.tensor_add(out=gates[:], in0=gates[:], in1=bt[:])
                    si = stream.tile([128, 128], FP, tag="lstm_si")
                    tg = stream.tile([128, 128], FP, tag="lstm_tg")
                    so = stream.tile([128, 128], FP, tag="lstm_so")
                    nc.scalar.activation(out=si[:], in_=gates[:, 0:128], func=mybir.ActivationFunctionType.Sigmoid)
                    nc.scalar.activation(out=tg[:], in_=gates[:, 256:384], func=mybir.ActivationFunctionType.Tanh)
                    nc.scalar.activation(out=so[:], in_=gates[:, 384:512], func=mybir.ActivationFunctionType.Sigmoid)
                    cc = stream.tile([128, 128], FP, tag="lstm_c")
                    nc.vector.tensor_mul(out=cc[:], in0=si[:], in1=tg[:])
                    tc_ = stream.tile([128, 128], FP, tag="lstm_tc")
                    nc.scalar.activation(out=tc_[:], in_=cc[:], func=mybir.ActivationFunctionType.Tanh)
                    nc.vector.tensor_mul(out=w_cur[:], in0=so[:], in1=tc_[:])
                return w_cur

            W0T = lstm_chain(w00_d, wih0_d, bih0_d, bhh0_d, "0")
            W1T = lstm_chain(w01_d, wih1_d, bih1_d, bhh1_d, "1")

            # ---------------- degree scales ----------------------------------
            def rsqrt_tile(src_ap, cols, tag):
                t = cst.tile([128, cols], FP, tag=f"rs_{tag}")
                nc.sync.dma_start(out=t[:], in_=src_ap)
                nc.vector.tensor_scalar_max(out=t[:], in0=t[:], scalar1=1.0)
                nc.vector.reciprocal(out=t[:], in_=t[:])
                nc.scalar.activation(out=t[:], in_=t[:], func=mybir.ActivationFunctionType.Sqrt)
                return t

            rs_out_g = rsqrt_tile(deg_out_g[:], nblk_glob, "og")
            rs_in_l = rsqrt_tile(deg_in_l[:], nblk, "il")
            rs_out_l = rsqrt_tile(deg_out_l[:], nblk, "ol")
            s_comb = cst.tile([128, nblk], FP, tag="s_comb")
            nc.vector.tensor_mul(out=s_comb[:], in0=rs_in_l[:], in1=rs_out_l[:])

            # ---------------- stage A: full permuted h0p table ---------------
            h0p = dpool.tile([NPOS, F], FP)
            nb_total = NPOS // 128
            c = 0
            while c * 4 < nb_total:
                bs = min(4, nb_total - c * 4)
                cw = bs * 128
                ft = stream.tile([128, 512], FP, tag="ft")
                nc.sync.dma_start(out=ft[:, :cw], in_=featsT[:, c * 512:c * 512 + cw])
                ot = stream.tile([128, 512], FP, tag="aout")
                for b in range(bs):
                    jg = c * 4 + b
                    pm = psA.tile([128, 128], FP, tag="t128")
                    nc.tensor.matmul(out=pm[:], lhsT=ft[:, b * 128:(b + 1) * 128], rhs=W0T[:],
                                     start=True, stop=True)
                    nc.vector.tensor_scalar(out=ot[:, b * 128:(b + 1) * 128], in0=pm[:],
                                            scalar1=rs_out_g[:, jg:jg + 1], scalar2=None,
                                            op0=mybir.AluOpType.mult)
                nc.sync.dma_start(
                    out=h0p[c * 512:c * 512 + cw, :].rearrange("(b p) f -> p b f", p=128),
                    in_=ot[:, :cw].rearrange("p (b f) -> p b f", f=128),
                )
                if debug:
                    nc.sync.dma_start(
                        out=dbg_h0p[c * 512:c * 512 + cw, :].rearrange("(b p) f -> p b f", p=128),
                        in_=ot[:, :cw].rearrange("p (b f) -> p b f", f=128),
                    )
                c += 1

            # ---------------- shared gather indices --------------------------
            idx_sb = main.tile([128, ctot16], I16, tag="idx")
            nc.sync.dma_start(out=idx_sb[:], in_=idx_d[:])

            # ---------------- conv aggregation -------------------------------
            ag_in = dpool.tile([nloc_pad, F], FP)
            ag_out = dpool.tile([NPOS, F], FP, addr_space="Shared")

            def conv(table, scale_tile):
                tabA = table[0:B0, :]
                tabB = table[B0:NPOS, :]
                aggs = []
                for g in range(ngrp):
                    agg = main.tile([128, grp_cols[g]], FP, tag=f"agg{g}")
                    nc.vector.memset(agg[:], 0.0)
                    aggs.append(agg)
                maxr = max(r for (_, _, r, _, _, _) in rounds) + 1
                by_r = {}
                for rd in rounds:
                    by_r.setdefault(rd[2], []).append(rd)
                for r in range(maxr):
                    for (half, g, rr, js_, je_, so) in by_r.get(r, []):
                        span = je_ - js_
                        S = span * 8
                        tab = tabA if half == "A" else tabB
                        if half == "A" and rr == 0:
                            dst = aggs[g][:, js_ * 128:je_ * 128]
                            nc.gpsimd.dma_gather(
                                dst.rearrange("p (j f) -> p j f", f=128),
                                tab, idx_sb[:, so:so + S],
                                span * 128, span * 128, F)
                        else:
                            gb = gpool.tile([128, GB * 128], FP, tag="gbuf")
                            nc.gpsimd.dma_gather(
                                gb[:, :span * 128].rearrange("p (j f) -> p j f", f=128),
                                tab, idx_sb[:, so:so + S],
                                span * 128, span * 128, F)
                            nc.vector.tensor_add(
                                out=aggs[g][:, js_ * 128:je_ * 128],
                                in0=aggs[g][:, js_ * 128:je_ * 128],
                                in1=gb[:, :span * 128])
                # epilogue per group: rrelu then per-node scale
                for g in range(ngrp):
                    gc = grp_cols[g]
                    tmp = main.tile([128, GB * 128], FP, tag="rrelu_tmp")
                    nc.vector.tensor_scalar(out=tmp[:, :gc], in0=aggs[g][:], scalar1=0.0,
                                            scalar2=1.0 - RRELU_SLOPE,
                                            op0=mybir.AluOpType.max, op1=mybir.AluOpType.mult)
                    nc.vector.scalar_tensor_tensor(out=aggs[g][:], in0=aggs[g][:], scalar=RRELU_SLOPE,
                                                   in1=tmp[:, :gc], op0=mybir.AluOpType.mult,
                                                   op1=mybir.AluOpType.add)
                    for jj in range(gc // 128):
                        j = g * GB + jj
                        nc.vector.tensor_scalar(out=aggs[g][:, jj * 128:(jj + 1) * 128],
                                                in0=aggs[g][:, jj * 128:(jj + 1) * 128],
                                                scalar1=scale_tile[:, j:j + 1], scalar2=None,
                                                op0=mybir.AluOpType.mult)
                return aggs

            aggs0 = conv(h0p, s_comb)
            if debug:
                for g in range(ngrp):
                    nc.sync.dma_start(out=dbg_agg0[:, g * GB * 128:g * GB * 128 + grp_cols[g]],
                                      in_=aggs0[g][:])

            # zero the pad rows of ag_in once
            zt = stream.tile([128, F], FP, tag="zt")
            nc.vector.memset(zt[:], 0.0)
            if nloc_pad > nloc:
                nc.sync.dma_start(out=ag_in[nloc:nloc_pad, :], in_=zt[:nloc_pad - nloc, :])

            for j in range(nblk):
                rows = min(128, nloc - j * 128)
                if rows <= 0:
                    break
                blkap = aggs0[j // GB][:, (j % GB) * 128:(j % GB) * 128 + 128]
                pt = psA.tile([128, 128], FP, tag="t128")
                nc.tensor.transpose(out=pt[:], in_=blkap, identity=ident[:])
                xT = stream.tile([128, 128], FP, tag="p_xT")
                nc.vector.tensor_copy(out=xT[:], in_=pt[:])
                pm = psA.tile([128, 128], FP, tag="t128")
                nc.tensor.matmul(out=pm[:], lhsT=xT[:], rhs=W1T[:], start=True, stop=True)
                ob = stream.tile([128, 128], FP, tag="p_ob")
                nc.vector.tensor_copy(out=ob[:], in_=pm[:])
                nc.sync.dma_start(out=ag_in[j * 128:j * 128 + rows, :], in_=ob[:rows, :])
                if debug:
                    nc.sync.dma_start(out=dbg_agin[j * 128:j * 128 + rows, :], in_=ob[:rows, :])

            nc.gpsimd.collective_compute(
                "AllGather", mybir.AluOpType.bypass,
                replica_groups=[list(range(n_cores))],
                ins=[ag_in[:].opt()], outs=[ag_out[:].opt()],
            )

            aggs1 = conv(ag_out, rs_in_l)
            if debug:
                for g in range(ngrp):
                    nc.sync.dma_start(out=dbg_agg1[:, g * GB * 128:g * GB * 128 + grp_cols[g]],
                                      in_=aggs1[g][:])

            # ---------------- MLP --------------------------------------------
            w1_t = cst.tile([128, CH], FP, tag="w1")
            nc.sync.dma_start(out=w1_t[:], in_=w1_d[:])
            b1_t = cst.tile([128, 3], FP, tag="b1")
            nc.sync.dma_start(out=b1_t[:], in_=b1_d[:])
            w2_t = cst.tile([128, 3 * NCLS], FP, tag="w2")
            nc.sync.dma_start(out=w2_t[:], in_=w2_d[:])
            b2_t = cst.tile([NCLS, 1], FP, tag="b2")
            nc.sync.dma_start(out=b2_t[:], in_=b2_d[:])

            out_sb = main.tile([NCLS, nloc_pad], FP, tag="out_sb")
            ch_off = [0, 128, 256]
            for cch in range((nblk + 3) // 4):
                bs = min(4, nblk - cch * 4)
                W = bs * 128
                xT = stream.tile([128, 512], FP, tag="m_xT")
                for b in range(bs):
                    j = cch * 4 + b
                    blkap = aggs1[j // GB][:, (j % GB) * 128:(j % GB) * 128 + 128]
                    pt = psA.tile([128, 128], FP, tag="t128")
                    nc.tensor.transpose(out=pt[:], in_=blkap, identity=ident[:])
                    nc.vector.tensor_copy(out=xT[:, b * 128:(b + 1) * 128], in_=pt[:])
                po = psC.tile([NCLS, 512], FP, tag="out2")
                for ci, (co, cs) in enumerate(zip(ch_off, ch_sizes)):
                    ph = psB.tile([128, 512], FP, tag="mm512")
                    nc.tensor.matmul(out=ph[:cs, :W], lhsT=w1_t[:, co:co + cs], rhs=xT[:, :W],
                                     start=True, stop=True)
                    hT = stream.tile([128, 512], FP, tag="m_hT")
                    nc.scalar.activation(out=hT[:cs, :W], in_=ph[:cs, :W],
                                         func=mybir.ActivationFunctionType.Relu,
                                         bias=b1_t[:cs, ci:ci + 1])
                    nc.tensor.matmul(out=po[:, :W], lhsT=w2_t[:cs, ci * NCLS:(ci + 1) * NCLS],
                                     rhs=hT[:cs, :W], start=(ci == 0), stop=(ci == 2))
                nc.vector.tensor_scalar(out=out_sb[:, cch * 512:cch * 512 + W], in0=po[:, :W],
                                        scalar1=b2_t[:, 0:1], scalar2=None,
                                        op0=mybir.AluOpType.add)
            nc.sync.dma_start(out=out_d[:], in_=out_sb[:])

    input_names = ["featsT", "idx", "deg_out_g", "deg_in_l", "deg_out_l",
                   "w00", "w01", "wih0", "wih1", "bih0", "bhh0", "bih1", "bhh1",
                   "mlp_w1", "mlp_b1", "mlp_w2", "mlp_b2"]
    return nc, input_names


# ----------------------------------------------------------------------------
# Host staging / unstaging
# ----------------------------------------------------------------------------

def _make_in_maps(inputs, sched):
    N = sched["n_nodes"]
    n_cores = sched["n_cores"]
    NPOS = sched["NPOS"]
    nloc = sched["nloc"]
    nloc_pad = sched["nloc_pad"]
    nblk_glob = NPOS // 128

    node_at_pos = np.full(NPOS, -1, np.int64)
    for k in range(n_cores):
        node_at_pos[k * nloc_pad: k * nloc_pad + nloc] = sched["local_nodes"][k]
    valid = node_at_pos >= 0

    feats_last = np.asarray(inputs["feats"][-1], np.float32)
    featsT = np.zeros((F, NPOS), np.float32)
    featsT[:, valid] = feats_last.T[:, node_at_pos[valid]]

    dof = np.ones(NPOS, np.float32)
    dof[valid] = sched["deg_out_full"][node_at_pos[valid]].astype(np.float32)
    deg_out_g = dof.reshape(nblk_glob, 128).T.copy()

    def pack_wih(w):
        return np.ascontiguousarray(
            w.reshape(4, 128, H).transpose(1, 0, 2).reshape(128, 4 * H)).astype(np.float32)

    def rep(b):
        return np.ascontiguousarray(np.broadcast_to(np.asarray(b, np.float32), (128, 4 * H)))

    b1 = np.asarray(inputs["mlp_b1"], np.float32)
    b1_t = np.zeros((128, 3), np.float32)
    w2 = np.asarray(inputs["mlp_w2"], np.float32)
    w2_t = np.zeros((128, 3 * NCLS), np.float32)
    for c, (co, cs) in enumerate(zip([0, 128, 256], [128, 128, CH - 256])):
        b1_t[:cs, c] = b1[co:co + cs]
        w2_t[:cs, c * NCLS:(c + 1) * NCLS] = w2[co:co + cs, :]

    common = dict(
        featsT=featsT,
        deg_out_g=deg_out_g,
        w00=np.asarray(inputs["W00"], np.float32),
        w01=np.asarray(inputs["W01"], np.float32),
        wih0=pack_wih(np.asarray(inputs["lstm0_wih"], np.float32)),
        wih1=pack_wih(np.asarray(inputs["lstm1_wih"], np.float32)),
        bih0=rep(inputs["lstm0_bih"]), bhh0=rep(inputs["lstm0_bhh"]),
        bih1=rep(inputs["lstm1_bih"]), bhh1=rep(inputs["lstm1_bhh"]),
        mlp_w1=np.asarray(inputs["mlp_w1"], np.float32),
        mlp_b1=b1_t,
        mlp_w2=w2_t,
        mlp_b2=np.asarray(inputs["mlp_b2"], np.float32).reshape(NCLS, 1),
    )
    in_maps = []
    for k in range(n_cores):
        m = dict(common)
        m["idx"] = sched["idx"][k]
        m["deg_in_l"] = sched["deg_in_loc"][k]
        m["deg_out_l"] = sched["deg_out_loc"][k]
        in_maps.append(m)
    return in_maps


def _assemble_output(results, sched):
    n_cores = sched["n_cores"]
    nloc = sched["nloc"]
    N = sched["n_nodes"]
    outs = [np.asarray(results[k]["out"])[:, :nloc] for k in range(n_cores)]
    flat = np.concatenate(outs, axis=1).T
    final = np.empty((N, NCLS), np.float32)
    node_at_pos = np.concatenate([sched["local_nodes"][k] for k in range(n_cores)])
    final[node_at_pos] = flat
    return final


# ----------------------------------------------------------------------------
# Entry point
# ----------------------------------------------------------------------------

LAST_RUN = {}
_CACHE = {}


def _prepare(inputs):
    src = np.asarray(inputs["src"][-1], np.int64)
    dst = np.asarray(inputs["dst"][-1], np.int64)
    n = int(np.asarray(inputs["feats"]).shape[1])
    key = (n, hash(src.tobytes()), hash(dst.tobytes()))
    if _CACHE.get("key") != key:
        sched = _build_schedule(src, dst, n, N_CORES, grp_blocks=7)
        nc, names = _build_program(sched)
        nc.compile()
        _CACHE.update(key=key, sched=sched, nc=nc, names=names)
    sched, nc = _CACHE["sched"], _CACHE["nc"]
    in_maps = _make_in_maps(inputs, sched)
    return sched, nc, in_maps


def kernel(**inputs):
    sched, nc, in_maps = _prepare(inputs)
    res = bass_utils.run_bass_kernel_spmd(
        nc, in_maps, core_ids=list(range(N_CORES)),
    )
    LAST_RUN["res"] = res
    LAST_RUN["sched"] = sched
    return _assemble_output(res.results, sched)


# ----------------------------------------------------------------------------
# Timing (dev utility; mirrors bass2jax.run_bass_via_pjrt but keeps inputs
# resident on device and times repeated executions)
# ----------------------------------------------------------------------------

def _make_runner(nc, n_cores):
    import jax
    from jax.sharding import Mesh, PartitionSpec
    from jax.experimental.shard_map import shard_map
    from concourse import bass2jax

    bass2jax.install_neuronx_cc_hook()
    partition_name = nc.partition_id_tensor.name if nc.partition_id_tensor else None
    in_names, out_names, out_avals, zero_outs = [], [], [], []
    for alloc in nc.m.functions[0].allocations:
        if not isinstance(alloc, mybir.MemoryLocationSet):
            continue
        name = alloc.memorylocations[0].name
        if alloc.kind == "ExternalInput":
            if name != partition_name:
                in_names.append(name)
        elif alloc.kind == "ExternalOutput":
            shape = tuple(alloc.tensor_shape)
            dtype = mybir.dt.np(alloc.dtype)
            out_names.append(name)
            out_avals.append(jax.core.ShapedArray(shape, dtype))
            zero_outs.append(np.zeros(shape, dtype))
    n_params = len(in_names)
    all_in = list(in_names) + list(out_names)
    if partition_name is not None:
        all_in.append(partition_name)

    def _body(*args):
        operands = list(args)
        if partition_name is not None:
            operands.append(bass2jax.partition_id_tensor())
        outs = bass2jax._bass_exec_p.bind(
            *operands,
            out_avals=tuple(out_avals),
            in_names=tuple(all_in),
            out_names=tuple(out_names),
            lowering_input_output_aliases=(),
            sim_require_finite=True,
            sim_require_nnan=True,
            nc=nc,
        )
        return tuple(outs)

    devices = jax.devices()[:n_cores]
    mesh = Mesh(np.asarray(devices), ("core",))
    n_outs = len(out_names)
    fn = jax.jit(
        shard_map(_body, mesh=mesh,
                  in_specs=(PartitionSpec("core"),) * (n_params + n_outs),
                  out_specs=(PartitionSpec("core"),) * n_outs,
                  check_rep=False),
        donate_argnums=tuple(range(n_params, n_params + n_outs)),
        keep_unused=True,
    )
    return fn, mesh, in_names, zero_outs


def _bench_nc(nc, in_maps, n_cores, iters):
    import time as _time
    import jax
    from jax.sharding import NamedSharding, PartitionSpec

    fn, mesh, in_names, zero_outs = _make_runner(nc, n_cores)
    shard = NamedSharding(mesh, PartitionSpec("core"))
    concat_in = [
        jax.device_put(
            np.concatenate([np.asarray(in_maps[c][n]) for c in range(n_cores)], axis=0),
            shard)
        for n in in_names
    ]

    def fresh_zeros():
        return [jax.device_put(
            np.zeros((n_cores * z.shape[0], *z.shape[1:]), z.dtype), shard)
            for z in zero_outs]

    out = fn(*concat_in, *fresh_zeros())
    jax.block_until_ready(out)
    times = []
    for _ in range(iters):
        zs = fresh_zeros()
        jax.block_until_ready(zs)
        t0 = _time.perf_counter()
        out = fn(*concat_in, *zs)
        jax.block_until_ready(out)
        times.append(_time.perf_counter() - t0)
    return np.array(times)


def _null_program(n_cores):
    nc = bacc.Bacc("TRN2", target_bir_lowering=False, debug=False,
                   num_devices=n_cores)
    x = nc.dram_tensor("x", [128, 128], FP, kind="ExternalInput")
    y = nc.dram_tensor("y", [128, 128], FP, kind="ExternalOutput")
    with tile.TileContext(nc) as tc:
        with tc.tile_pool(name="sb", bufs=1) as sb:
            t = sb.tile([128, 128], FP)
            nc.sync.dma_start(out=t[:], in_=x[:])
            nc.sync.dma_start(out=y[:], in_=t[:])
    nc.compile()
    return nc, [{"x": np.zeros((128, 128), np.float32)} for _ in range(n_cores)]


def benchmark(inputs, iters=8):
    sched, nc, in_maps = _prepare(inputs)
    t_kernel = _bench_nc(nc, in_maps, N_CORES, iters)
    nc0, im0 = _null_program(N_CORES)
    t_null = _bench_nc(nc0, im0, N_CORES, iters)
    k_med = float(np.median(t_kernel))
    n_med = float(np.median(t_null))
    print(f"kernel call: median {k_med*1e6:.0f}us (min {t_kernel.min()*1e6:.0f}us); "
          f"null call: median {n_med*1e6:.0f}us (min {t_null.min()*1e6:.0f}us)")
    return max(0.0, (k_med - n_med)) * 1e9
